# revision 19
# baseline (speedup 1.0000x reference)
"""Multi-head attention (B=4, N=2048, C=1024, H=16) on 8 TRN2 NeuronCores.

Sharding: core = 2*b + half handles batch b, heads half*8 .. half*8+7.
Each core computes QKV for its 8 heads, full attention for them, and a
partial projection (its 512 rows of W_proj). Host sums the two partials
per batch and adds the bias.

All matmul operands are fp16 (1 cycle/row on the PE); accumulation stays
fp32 in PSUM. The host pre-casts weights/x to fp16 and pre-transposes x.

On-chip layout is "transposed": Q^T/K^T [d, n] come straight out of the
QKV matmuls, scores are computed as S^T[m, n] so exp(S^T) = P^T is the
moving operand of the AV matmul. Row sums of P ride along as a 65th
stationary column of ones.

v2 pipeline (vs the v1 baseline at 577us):
- Softmax normalization is DEFERRED by two nb-iterations and runs entirely
  off the tensor queue: DVE reciprocal of the raw row sums, a GPSIMD
  partition_broadcast replacing the K=1 broadcast matmuls, two DVE
  multiplies, and the head-B partition-shift DMA. The PE never waits on
  the (slow, 8 cyc/elem) DVE reciprocal.
- All weights are prefetched up front; the exp activation table is
  preloaded during the input DMA.
- Q^T/K^T for pair p+1 are computed one matmul per m-iteration inside
  pair p's ACT-paced m-loop (PE slack), as is the projection during the
  last pair's m-loops.
- Output is fp16 (halves the tail DMA); host accumulates in fp32.
"""

import functools
from contextlib import ExitStack

import numpy as np

import concourse.bass as bass
import concourse.tile as tile
from concourse import bacc, mybir
from concourse.bass_utils import run_bass_kernel_spmd

F32 = mybir.dt.float32
F16 = mybir.dt.float16
AF = mybir.ActivationFunctionType

B, N, C = 4, 2048, 1024
H, D = 16, 64
P = 128
NCORES = 8
HPC = 8            # heads per core
PAIRS = HPC // 2   # 4
DCORE = HPC * D    # 512 attention columns per core
SCALE = float(H) ** -0.5  # 0.25 (faithful to reference: num_heads**-0.5)
EXP_BIAS = -5.0    # exp(scale*s + bias): cancels in softmax, keeps fp16 range
NB = N // 512      # 4 n blocks
NT = N // P        # 16 tiles of 128
CT = C // P        # 8 contraction chunks
VW = D + 1         # V columns per head incl. the ones column (row sums)
MBLK = HPC * VW    # 520 v_sb columns per m-tile

LAST_RESULT = None  # BassKernelResults of the most recent run (for test.py)


def _kernel_body(tc, out_d, xt_d, wqk_d, wv_d, wp_d):
    nc = tc.nc
    with ExitStack() as ctx:
        const = ctx.enter_context(tc.tile_pool(name="const", bufs=1))
        ones_f = const.tile([P, P], F32)
        nc.vector.memset(ones_f, 1.0)
        ebias = const.tile([P, 1], F32)
        nc.vector.memset(ebias, EXP_BIAS)
        warm = const.tile([P, 1], F16)

        # attT: pair p occupies cols [p*N, (p+1)*N); partitions = 2 heads x 64
        attT_pool = ctx.enter_context(tc.tile_pool(name="attT", bufs=1))
        attT = attT_pool.tile([P, PAIRS * N], F16)

        # PSUM: ps_mm 2 (V/QK/proj chains) + ps_s 2x2 (scores) + av 2 = 8 banks
        ps_mm = ctx.enter_context(tc.tile_pool(name="ps_mm", bufs=2, space="PSUM"))
        ps_s = ctx.enter_context(tc.tile_pool(name="ps_s", bufs=2, space="PSUM"))
        ps_av = ctx.enter_context(tc.tile_pool(name="ps_av", bufs=1, space="PSUM"))

        # ---- Phase 0: prefetch everything; preload the exp table ----
        w_pool = ctx.enter_context(tc.tile_pool(name="weights", bufs=1))
        wv_sb = w_pool.tile([P, CT * DCORE], F16)
        xt_pool = ctx.enter_context(tc.tile_pool(name="xt", bufs=1))
        xt = xt_pool.tile([P, CT * N], F16)
        # wqk layout: tensor t (0=q, 1=k), pair p at [(t*PAIRS+p) * CT*P, +CT*P)
        wqk_sb = w_pool.tile([P, 2 * PAIRS * CT * P], F16)
        wp_sb = w_pool.tile([P, PAIRS * C], F16)

        # All weights are host-repacked so every partition's data is one
        # contiguous DRAM run (8-32KB lines -> full HBM bandwidth).
        # Order: wv, xt chunks (V-phase gating), wqk, wp.
        nc.sync.dma_start(out=wv_sb, in_=wv_d)
        for j in range(CT):
            nc.sync.dma_start(out=xt[:, j * N:(j + 1) * N],
                              in_=xt_d[j * P:(j + 1) * P, :])
        nc.sync.dma_start(out=wqk_sb, in_=wqk_d)
        nc.sync.dma_start(out=wp_sb, in_=wp_d)
        # exp table preload (~2.7us) overlaps the input DMA
        nc.scalar.activation(warm, ebias, AF.Exp)

        # V storage: m-tile m at cols [m*MBLK, ...); head hl at
        # [m*MBLK + hl*VW, +D], then a ones column (for row sums)
        v_pool = ctx.enter_context(tc.tile_pool(name="v", bufs=1))
        v_sb = v_pool.tile([P, NT * MBLK], F16)
        ones_cols = v_sb.rearrange("q (g k) -> q g k", k=VW)[:, :, D:VW]
        nc.vector.tensor_copy(
            ones_cols, ones_f.rearrange("q (g k) -> q g k", k=1))

        # ---- Phase B1: V for all 8 heads ----
        for m in range(NT):
            psv = ps_mm.tile([P, DCORE], F32, tag="mm")
            for cc in range(CT):
                nc.tensor.matmul(
                    psv,
                    xt[:, cc * N + m * P: cc * N + (m + 1) * P],
                    wv_sb[:, cc * DCORE:(cc + 1) * DCORE],
                    start=(cc == 0), stop=(cc == CT - 1))
            nc.vector.tensor_copy(
                v_sb[:, m * MBLK:(m + 1) * MBLK].rearrange(
                    "q (h k) -> q h k", k=VW)[:, :, 0:D],
                psv.rearrange("q (h k) -> q h k", k=D))

        # ---- QK + attention, software-pipelined across pairs ----
        qt_pool = ctx.enter_context(tc.tile_pool(name="qt", bufs=2))
        kt_pool = ctx.enter_context(tc.tile_pool(name="kt", bufs=2))
        pt_pool = ctx.enter_context(tc.tile_pool(name="pt", bufs=4))
        rc_pool = ctx.enter_context(tc.tile_pool(name="rc", bufs=3))
        rc0_pool = ctx.enter_context(tc.tile_pool(name="rc0", bufs=3))
        rbt_pool = ctx.enter_context(tc.tile_pool(name="rbt", bufs=3))
        tmb_pool = ctx.enter_context(tc.tile_pool(name="tmb", bufs=4))
        stage_pool = ctx.enter_context(tc.tile_pool(name="stage", bufs=3))

        def new_qk_tiles():
            qt = qt_pool.tile([P, N], F16, tag="qt")
            kt = kt_pool.tile([P, N], F16, tag="kt")
            return qt, kt

        def qk_mm_list(p, qt, kt):
            """Flat (chain-structured) QK matmul emitters for pair p.

            8 chains of 8: chain j = (tensor t=j//4, nb j%4); each chain
            accumulates CT chunks into a ps_mm tile then evicts to qt/kt.
            Returns a list of 64 closures, one matmul (+ trailing copy) each.
            """
            state = {}

            def make(j, cc):
                def emit():
                    t, nbq = j // 4, j % 4
                    if cc == 0:
                        state[j] = ps_mm.tile([P, 512], F32, tag="mm", name=f"psq{j}")
                    psq = state[j]
                    wbase = (t * PAIRS + p) * CT * P
                    nc.tensor.matmul(
                        psq,
                        wqk_sb[:, wbase + cc * P: wbase + (cc + 1) * P],
                        xt[:, cc * N + nbq * 512: cc * N + nbq * 512 + 512],
                        start=(cc == 0), stop=(cc == CT - 1),
                        skip_group_check=True)
                    if cc == CT - 1:
                        dst = qt if t == 0 else kt
                        nc.vector.tensor_copy(
                            dst[:, nbq * 512:(nbq + 1) * 512], psq)
                        del state[j]
                return emit
            return [make(j, cc) for j in range(8) for cc in range(CT)]

        def proj_mm_list(k, scalar_evict=False):
            """Projection chunk k: i-tiles 4k..4k+3, both co halves.

            Chains of 4 dc-matmuls into ps_mm, then fp16 stage + DMA out.
            32 closures, one matmul (+ trailing evict) each. At the tail
            the exp stream is over, so the stage copies route to the idle
            ScalarE instead of the busy DVE (scalar_evict=True).
            """
            state = {}

            def make(i, co, dc):
                def emit():
                    key = (i, co)
                    if dc == 0:
                        state[key] = ps_mm.tile([P, 512], F32, tag="mm", name=f"psp{i}_{co}")
                    psp = state[key]
                    nc.tensor.matmul(
                        psp,
                        attT[:, dc * N + i * P: dc * N + (i + 1) * P],
                        wp_sb[:, dc * C + co * 512: dc * C + co * 512 + 512],
                        start=(dc == 0), stop=(dc == PAIRS - 1),
                        skip_group_check=True)
                    if dc == PAIRS - 1:
                        st = stage_pool.tile([P, 512], F16, tag="st")
                        if scalar_evict:
                            nc.scalar.copy(st, psp)
                        else:
                            nc.vector.tensor_copy(st, psp)
                        nc.sync.dma_start(
                            out=out_d[i * P:(i + 1) * P,
                                      co * 512: co * 512 + 512],
                            in_=st)
                        del state[key]
                return emit
            return [make(i, co, dc)
                    for i in range(4 * k, 4 * k + 4)
                    for co in range(2)
                    for dc in range(PAIRS)]

        def stage_a(pend):
            """Deferred norm 1/2: reciprocal (DVE) + partition broadcast
            (GPSIMD daisy chain) — both off the tensor queue. The Q7
            broadcast ucode reads partition 0, hence the rc0 shift."""
            with nc.allow_low_precision(
                    reason="softmax recip rounding is benign"):
                nc.vector.reciprocal(pend["rc0"][0:1, :], pend["rc0"][0:1, :])
            rbt = rbt_pool.tile([64, 1024], F16, tag="rbt")
            nc.gpsimd.partition_broadcast(rbt, pend["rc0"][0:1, :])
            pend["rbt"] = rbt

        def stage_b(pend):
            """Deferred norm 2/2: scale by the broadcast reciprocals, then
            shift head B's rows (partitions 0-63) up to partitions 64-127."""
            rbt, tmb, osl = pend["rbt"], pend["tmb"], pend["osl"]
            nc.vector.tensor_mul(attT[0:64, osl], attT[0:64, osl],
                                 rbt[:, 0:512])
            nc.vector.tensor_mul(tmb, tmb, rbt[:, 512:1024])
            nc.sync.dma_start(out=attT[64:128, osl], in_=tmb)

        # QK for pair 0 runs serially (nothing to hide it under)
        qk_cur = new_qk_tiles()
        for emit in qk_mm_list(0, *qk_cur):
            emit()

        pend_a = None   # evicted, awaiting stage_a
        pend_b = None   # awaiting stage_b
        qk_next = None
        for p in range(PAIRS):
            qt, kt = qk_cur
            if p < PAIRS - 1:
                qk_next = new_qk_tiles()
                bg = qk_mm_list(p + 1, *qk_next)   # 64 mms: 2/iter, front-half
                bg_per_iter = 2
            else:
                bg = []                            # filled per-nb with proj
                bg_per_iter = 3
            bgi = 0

            for nb in range(NB):
                if pend_b is not None:
                    stage_b(pend_b)
                    pend_b = None
                if pend_a is not None:
                    stage_a(pend_a)
                    pend_b = pend_a
                    pend_a = None
                if p == PAIRS - 1 and nb >= 2:
                    # proj chunk nb-2 became ready with stage_b(p, nb-2)
                    bg = bg + proj_mm_list(nb - 2)

                nsl = slice(nb * 512, nb * 512 + 512)
                osl = slice(p * N + nb * 512, p * N + nb * 512 + 512)
                ps_av_a = ps_av.tile([P, 512], F32, tag="avA")
                ps_av_b = ps_av.tile([P, 512], F32, tag="avB")
                pt_prev = None
                m_prev = None
                for m in range(NT):
                    ps_s_t = ps_s.tile([P, 1024], F32, tag="s")
                    nc.tensor.matmul(
                        ps_s_t[:, 0:512],
                        kt[0:64, m * P:(m + 1) * P],
                        qt[0:64, nsl],
                        start=True, stop=True, skip_group_check=True)
                    nc.tensor.matmul(
                        ps_s_t[:, 512:1024],
                        kt[64:128, m * P:(m + 1) * P],
                        qt[64:128, nsl],
                        start=True, stop=True, skip_group_check=True)
                    pt = pt_pool.tile([P, 1024], F16, tag="pt")
                    nc.scalar.activation(pt, ps_s_t, AF.Exp,
                                         scale=SCALE, bias=ebias)
                    # background PE work (QK of next pair / projection),
                    # front-loaded 2/iter so the last chain's eviction
                    # lands well before the pair boundary; proj waits 2
                    # iters so stage_b's muls land first
                    if p < PAIRS - 1 or m >= 2:
                        for _ in range(bg_per_iter):
                            if bgi < len(bg):
                                bg[bgi]()
                                bgi += 1
                    # AV trails scores by one m-iteration (softer PSUM reuse)
                    if pt_prev is not None:
                        self_m = m_prev
                        vbase = self_m * MBLK + 2 * p * VW
                        nc.tensor.matmul(
                            ps_av_a[0:VW, :],
                            v_sb[:, vbase: vbase + VW],
                            pt_prev[:, 0:512],
                            start=(self_m == 0), stop=False,
                            skip_group_check=True)
                        nc.tensor.matmul(
                            ps_av_b[0:VW, :],
                            v_sb[:, vbase + VW: vbase + 2 * VW],
                            pt_prev[:, 512:1024],
                            start=(self_m == 0), stop=False,
                            skip_group_check=True)
                    pt_prev, m_prev = pt, m
                vbase = m_prev * MBLK + 2 * p * VW
                nc.tensor.matmul(
                    ps_av_a[0:VW, :],
                    v_sb[:, vbase: vbase + VW],
                    pt_prev[:, 0:512],
                    start=False, stop=True, skip_group_check=True)
                nc.tensor.matmul(
                    ps_av_b[0:VW, :],
                    v_sb[:, vbase + VW: vbase + 2 * VW],
                    pt_prev[:, 512:1024],
                    start=False, stop=True, skip_group_check=True)

                # evict eagerly (free the PSUM banks); normalize lazily
                rc = rc_pool.tile([P, 1024], F16, tag="rc")
                nc.vector.tensor_copy(rc[64:65, 0:512], ps_av_a[D:VW, :])
                nc.vector.tensor_copy(rc[64:65, 512:1024], ps_av_b[D:VW, :])
                nc.vector.tensor_copy(attT[0:64, osl], ps_av_a[0:64, :])
                tmb = tmb_pool.tile([64, 512], F16, tag="tmb")
                nc.vector.tensor_copy(tmb, ps_av_b[0:64, :])
                # partition 64 -> 0 shift for the Q7 broadcast
                rc0 = rc0_pool.tile([1, 1024], F16, tag="rc0")
                nc.sync.dma_start(out=rc0, in_=rc[64:65, :])
                pend_a = {"rc0": rc0, "tmb": tmb, "osl": osl}

            qk_cur = qk_next

        # tail: final normalizations + remaining projection work.
        # stage_a(3,3)'s reciprocal runs on DVE concurrently with chunk 2's
        # matmuls; chunk evictions ride the now-idle ScalarE.
        stage_b(pend_b)                      # (3, 2)
        stage_a(pend_a)                      # (3, 3)
        for emit in proj_mm_list(2, scalar_evict=True):
            emit()
        stage_b(pend_a)                      # (3, 3)
        for emit in proj_mm_list(3, scalar_evict=True):
            emit()


@functools.lru_cache(maxsize=1)
def build_nc():
    nc = bacc.Bacc("TRN2", target_bir_lowering=False, debug=False)
    xt_d = nc.dram_tensor("xt_local", [C, N], F16, kind="ExternalInput").ap()
    wqk_d = nc.dram_tensor("wqk", [P, 2 * PAIRS * CT * P], F16,
                           kind="ExternalInput").ap()
    wv_d = nc.dram_tensor("wv", [P, CT * DCORE], F16, kind="ExternalInput").ap()
    wp_d = nc.dram_tensor("wp", [P, PAIRS * C], F16, kind="ExternalInput").ap()
    out_d = nc.dram_tensor("out_partial", [N, C], F16, kind="ExternalOutput").ap()
    with tile.TileContext(nc) as tc:
        _kernel_body(tc, out_d, xt_d, wqk_d, wv_d, wp_d)
    nc.compile()
    return nc


def make_in_maps(x, W_qkv, W_proj):
    """Shard + repack host-side. Weights land in SBUF-partition-major
    layout so each DMA reads long contiguous DRAM runs per partition:
      wv[q, cc*512+d]           = W_v[cc*128+q, d]
      wqk[q, ((t*4+p)*8+cc)*128+f] = W_{q|k}[cc*128+q, p*128+f]
      wp[q, dc*1024+c]          = W_proj[dc*128+q, c] (per-core rows)
    """
    in_maps = []
    for core in range(NCORES):
        b, half = core // 2, core % 2
        h0 = half * HPC
        wq = W_qkv[:, 0 * C + h0 * D: 0 * C + h0 * D + DCORE].astype(np.float16)
        wk = W_qkv[:, 1 * C + h0 * D: 1 * C + h0 * D + DCORE].astype(np.float16)
        wv = W_qkv[:, 2 * C + h0 * D: 2 * C + h0 * D + DCORE].astype(np.float16)
        wp = W_proj[h0 * D: h0 * D + DCORE, :].astype(np.float16)
        # [C, DCORE] -> per-pair column blocks, partition-major
        wqk = np.stack([w.reshape(CT, P, PAIRS, P).transpose(1, 2, 0, 3)
                        for w in (wq, wk)], axis=1)      # [q, t, p, cc, f]
        in_maps.append({
            "xt_local": np.ascontiguousarray(x[b].T.astype(np.float16)),
            "wqk": np.ascontiguousarray(wqk.reshape(P, 2 * PAIRS * CT * P)),
            "wv": np.ascontiguousarray(
                wv.reshape(CT, P, DCORE).transpose(1, 0, 2).reshape(P, CT * DCORE)),
            "wp": np.ascontiguousarray(
                wp.reshape(PAIRS, P, C).transpose(1, 0, 2).reshape(P, PAIRS * C)),
        })
    return in_maps


def kernel(x, W_qkv, W_proj, b_proj, trace=False):
    x = np.asarray(x, dtype=np.float32)
    W_qkv = np.asarray(W_qkv, dtype=np.float32)
    W_proj = np.asarray(W_proj, dtype=np.float32)
    b_proj = np.asarray(b_proj, dtype=np.float32)

    nc = build_nc()
    in_maps = make_in_maps(x, W_qkv, W_proj)

    global LAST_RESULT
    res = run_bass_kernel_spmd(nc, in_maps, list(range(NCORES)), trace=trace)
    LAST_RESULT = res

    out = np.empty((B, N, C), dtype=np.float32)
    for b in range(B):
        out[b] = (res.results[2 * b]["out_partial"].astype(np.float32)
                  + res.results[2 * b + 1]["out_partial"].astype(np.float32)
                  + b_proj[None, :])
    return out


# revision 25
# speedup vs baseline: 1.1795x; 1.1795x over previous
"""Multi-head attention (B=4, N=2048, C=1024, H=16) on 8 TRN2 NeuronCores.

Sharding: core = 2*b + half handles batch b, heads half*8 .. half*8+7.
Each core computes QKV for its 8 heads, full attention for them, and a
partial projection (its 512 rows of W_proj). Host sums the two partials
per batch and adds the bias.

All matmul operands are fp16 (1 cycle/row on the PE); accumulation stays
fp32 in PSUM. The host pre-casts weights/x to fp16 and pre-transposes x.

On-chip layout is "transposed": Q^T/K^T [d, n] come straight out of the
QKV matmuls, scores are computed as S^T[m, n] so exp(S^T) = P^T is the
moving operand of the AV matmul. Row sums of P ride along as a 65th
stationary column of ones.

v2 pipeline (vs the v1 baseline at 577us):
- Softmax normalization is DEFERRED by two nb-iterations and runs entirely
  off the tensor queue: DVE reciprocal of the raw row sums, a GPSIMD
  partition_broadcast replacing the K=1 broadcast matmuls, two DVE
  multiplies, and the head-B partition-shift DMA. The PE never waits on
  the (slow, 8 cyc/elem) DVE reciprocal.
- All weights are prefetched up front; the exp activation table is
  preloaded during the input DMA.
- Q^T/K^T for pair p+1 are computed one matmul per m-iteration inside
  pair p's ACT-paced m-loop (PE slack), as is the projection during the
  last pair's m-loops.
- Output is fp16 (halves the tail DMA); host accumulates in fp32.
"""

import functools
from contextlib import ExitStack

import numpy as np

import concourse.bass as bass
import concourse.tile as tile
from concourse import bacc, mybir
from concourse.bass_utils import run_bass_kernel_spmd

F32 = mybir.dt.float32
F16 = mybir.dt.float16
AF = mybir.ActivationFunctionType

B, N, C = 4, 2048, 1024
H, D = 16, 64
P = 128
NCORES = 8
HPC = 8            # heads per core
PAIRS = HPC // 2   # 4
DCORE = HPC * D    # 512 attention columns per core
SCALE = float(H) ** -0.5  # 0.25 (faithful to reference: num_heads**-0.5)
EXP_BIAS = -5.0    # exp(scale*s + bias): cancels in softmax, keeps fp16 range
NB = N // 512      # 4 n blocks
NT = N // P        # 16 tiles of 128
CT = C // P        # 8 contraction chunks
VW = D + 1         # V columns per head incl. the ones column (row sums)
MBLK = HPC * VW    # 520 v_sb columns per m-tile

LAST_RESULT = None  # BassKernelResults of the most recent run (for test.py)


def _kernel_body(tc, out_d, xt_d, wqk_d, wv_d, wp_d):
    nc = tc.nc
    with ExitStack() as ctx:
        const = ctx.enter_context(tc.tile_pool(name="const", bufs=1))
        ones_f = const.tile([P, P], F32)
        nc.vector.memset(ones_f, 1.0)
        ebias = const.tile([P, 1], F32)
        nc.vector.memset(ebias, EXP_BIAS)
        warm = const.tile([P, 1], F16)

        # attT: pair p occupies cols [p*N, (p+1)*N); partitions = 2 heads x 64
        attT_pool = ctx.enter_context(tc.tile_pool(name="attT", bufs=1))
        attT = attT_pool.tile([P, PAIRS * N], F16)

        # PSUM: ps_mm 2 (V/QK/proj chains) + ps_s 2x2 (scores) + av 2 = 8 banks
        ps_mm = ctx.enter_context(tc.tile_pool(name="ps_mm", bufs=2, space="PSUM"))
        ps_s = ctx.enter_context(tc.tile_pool(name="ps_s", bufs=2, space="PSUM"))
        ps_av = ctx.enter_context(tc.tile_pool(name="ps_av", bufs=1, space="PSUM"))

        # ---- Phase 0: prefetch everything; preload the exp table ----
        w_pool = ctx.enter_context(tc.tile_pool(name="weights", bufs=1))
        wv_sb = w_pool.tile([P, CT * DCORE], F16)
        xt_pool = ctx.enter_context(tc.tile_pool(name="xt", bufs=1))
        xt = xt_pool.tile([P, CT * N], F16)
        # wqk layout: tensor t (0=q, 1=k), pair p at [(t*PAIRS+p) * CT*P, +CT*P)
        wqk_sb = w_pool.tile([P, 2 * PAIRS * CT * P], F16)
        wp_sb = w_pool.tile([P, PAIRS * C], F16)

        # All weights are host-repacked so every partition's data is one
        # contiguous DRAM run (8-32KB lines). Inputs split across BOTH
        # HWDGE queues (SP + Activation) for ~2x landing bandwidth; wv and
        # xt chunks first (they gate the V phase), wqk/wp trail.
        half = CT * DCORE // 2
        nc.sync.dma_start(out=wv_sb[:, 0:half], in_=wv_d[:, 0:half])
        nc.scalar.dma_start(out=wv_sb[:, half:], in_=wv_d[:, half:])
        for j in range(CT):
            eng = nc.sync if j % 2 == 0 else nc.scalar
            eng.dma_start(out=xt[:, j * N:(j + 1) * N],
                          in_=xt_d[j * P:(j + 1) * P, :])
        nc.sync.dma_start(out=wqk_sb, in_=wqk_d)
        nc.scalar.dma_start(out=wp_sb, in_=wp_d)
        # exp table preload (~2.7us) overlaps the input DMA
        nc.scalar.activation(warm, ebias, AF.Exp)

        # V storage: m-tile m at cols [m*MBLK, ...); head hl at
        # [m*MBLK + hl*VW, +D], then a ones column (for row sums)
        v_pool = ctx.enter_context(tc.tile_pool(name="v", bufs=1))
        v_sb = v_pool.tile([P, NT * MBLK], F16)
        ones_cols = v_sb.rearrange("q (g k) -> q g k", k=VW)[:, :, D:VW]
        nc.vector.tensor_copy(
            ones_cols, ones_f.rearrange("q (g k) -> q g k", k=1))

        # ---- Phase B1: V for all 8 heads ----
        # 8 chains open at once (borrowing the idle ps_s/ps_av banks) so
        # 7/8 of the matmuls pre-run while the xt DMA is still landing;
        # only each chain's last chunk gates on the final xt arrival.
        def v_evict(psv_half, m):
            nc.vector.tensor_copy(
                v_sb[:, m * MBLK:(m + 1) * MBLK].rearrange(
                    "q (h k) -> q h k", k=VW)[:, :, 0:D],
                psv_half.rearrange("q (h k) -> q h k", k=D))

        for g in range(2):
            slots = [
                ps_mm.tile([P, DCORE], F32, tag="mm", name=f"vmm0_{g}"),
                ps_mm.tile([P, DCORE], F32, tag="mm", name=f"vmm1_{g}"),
                ps_av.tile([P, DCORE], F32, tag="avA", name=f"vavA_{g}"),
                ps_av.tile([P, DCORE], F32, tag="avB", name=f"vavB_{g}"),
                ps_s.tile([P, 2 * DCORE], F32, tag="s", name=f"vs0_{g}"),
                ps_s.tile([P, 2 * DCORE], F32, tag="s", name=f"vs1_{g}"),
            ]

            def v_dst(j):
                if j < 4:
                    return slots[j]
                big = slots[4 + (j - 4) // 2]
                h = (j - 4) % 2
                return big[:, h * DCORE:(h + 1) * DCORE]

            for cc in range(CT):
                for j in range(8):
                    m = 8 * g + j
                    nc.tensor.matmul(
                        v_dst(j),
                        xt[:, cc * N + m * P: cc * N + (m + 1) * P],
                        wv_sb[:, cc * DCORE:(cc + 1) * DCORE],
                        start=(cc == 0), stop=(cc == CT - 1),
                        skip_group_check=True)
            for j in range(8):
                v_evict(v_dst(j), 8 * g + j)

        # ---- QK + attention, software-pipelined across pairs ----
        qt_pool = ctx.enter_context(tc.tile_pool(name="qt", bufs=2))
        kt_pool = ctx.enter_context(tc.tile_pool(name="kt", bufs=2))
        pt_pool = ctx.enter_context(tc.tile_pool(name="pt", bufs=4))
        scr_pool = ctx.enter_context(tc.tile_pool(name="scr", bufs=3))
        rc0_pool = ctx.enter_context(tc.tile_pool(name="rc0", bufs=3))
        rbt_pool = ctx.enter_context(tc.tile_pool(name="rbt", bufs=3))
        stage_pool = ctx.enter_context(tc.tile_pool(name="stage", bufs=3))

        def new_qk_tiles():
            qt = qt_pool.tile([P, N], F16, tag="qt")
            kt = kt_pool.tile([P, N], F16, tag="kt")
            return qt, kt

        def qk_mm_list(p, qt, kt):
            """Flat (chain-structured) QK matmul emitters for pair p.

            8 chains of 8: chain j = (tensor t=j//4, nb j%4); each chain
            accumulates CT chunks into a ps_mm tile then evicts to qt/kt.
            Returns a list of 64 closures, one matmul (+ trailing copy) each.
            """
            state = {}

            def make(j, cc):
                def emit():
                    t, nbq = j // 4, j % 4
                    if cc == 0:
                        state[j] = ps_mm.tile([P, 512], F32, tag="mm", name=f"psq{j}")
                    psq = state[j]
                    wbase = (t * PAIRS + p) * CT * P
                    nc.tensor.matmul(
                        psq,
                        wqk_sb[:, wbase + cc * P: wbase + (cc + 1) * P],
                        xt[:, cc * N + nbq * 512: cc * N + nbq * 512 + 512],
                        start=(cc == 0), stop=(cc == CT - 1),
                        skip_group_check=True)
                    if cc == CT - 1:
                        dst = qt if t == 0 else kt
                        nc.vector.tensor_copy(
                            dst[:, nbq * 512:(nbq + 1) * 512], psq)
                        del state[j]
                return emit
            return [make(j, cc) for j in range(8) for cc in range(CT)]

        def proj_mm_list(k, scalar_evict=False):
            """Projection chunk k: i-tiles 4k..4k+3, both co halves.

            Chains of 4 dc-matmuls into ps_mm, then fp16 stage + DMA out.
            32 closures, one matmul (+ trailing evict) each. At the tail
            the exp stream is over, so the stage copies route to the idle
            ScalarE instead of the busy DVE (scalar_evict=True).
            """
            state = {}

            def make(i, co, dc):
                def emit():
                    key = (i, co)
                    if dc == 0:
                        state[key] = ps_mm.tile([P, 512], F32, tag="mm", name=f"psp{i}_{co}")
                    psp = state[key]
                    nc.tensor.matmul(
                        psp,
                        attT[:, dc * N + i * P: dc * N + (i + 1) * P],
                        wp_sb[:, dc * C + co * 512: dc * C + co * 512 + 512],
                        start=(dc == 0), stop=(dc == PAIRS - 1),
                        skip_group_check=True)
                    if dc == PAIRS - 1:
                        st = stage_pool.tile([P, 512], F16, tag="st")
                        if scalar_evict:
                            nc.scalar.copy(st, psp)
                        else:
                            nc.vector.tensor_copy(st, psp)
                        nc.sync.dma_start(
                            out=out_d[i * P:(i + 1) * P,
                                      co * 512: co * 512 + 512],
                            in_=st)
                        del state[key]
                return emit
            return [make(i, co, dc)
                    for i in range(4 * k, 4 * k + 4)
                    for co in range(2)
                    for dc in range(PAIRS)]

        def stage_a(pend):
            """Deferred norm 1/2: reciprocal (DVE) + partition broadcast
            (GPSIMD daisy chain) — both off the tensor queue. The Q7
            broadcast ucode reads partition 0, hence the rc0 shift."""
            with nc.allow_low_precision(
                    reason="softmax recip rounding is benign"):
                nc.vector.reciprocal(pend["rc0"][0:1, :], pend["rc0"][0:1, :])
            rbt = rbt_pool.tile([64, 1024], F16, tag="rbt")
            nc.gpsimd.partition_broadcast(rbt, pend["rc0"][0:1, :])
            pend["rbt"] = rbt

        def stage_b(pend):
            """Deferred norm 2/2: scale by the broadcast reciprocals
            (head A lands in attT directly), then shift head B's rows
            (partitions 0-63) up to partitions 64-127."""
            rbt, scra, scrb, osl = (pend["rbt"], pend["scra"],
                                    pend["scrb"], pend["osl"])
            nc.vector.tensor_mul(attT[0:64, osl], scra[0:64, :],
                                 rbt[:, 0:512])
            nc.vector.tensor_mul(scrb[0:64, :], scrb[0:64, :],
                                 rbt[:, 512:1024])
            nc.sync.dma_start(out=attT[64:128, osl], in_=scrb[0:64, :])

        # QK for pair 0 runs serially (nothing to hide it under)
        qk_cur = new_qk_tiles()
        for emit in qk_mm_list(0, *qk_cur):
            emit()

        pend_a = None   # evicted, awaiting stage_a
        pend_b = None   # awaiting stage_b
        qk_next = None
        for p in range(PAIRS):
            qt, kt = qk_cur
            if p < PAIRS - 1:
                qk_next = new_qk_tiles()
                bg = qk_mm_list(p + 1, *qk_next)   # 64 mms: 2/iter, front-half
                bg_per_iter = 2
            else:
                bg = []                            # filled per-nb with proj
                bg_per_iter = 3
            bgi = 0

            for nb in range(NB):
                if pend_b is not None:
                    stage_b(pend_b)
                    pend_b = None
                if pend_a is not None:
                    stage_a(pend_a)
                    pend_b = pend_a
                    pend_a = None
                if p == PAIRS - 1 and nb >= 2:
                    # proj chunk nb-2 became ready with stage_b(p, nb-2)
                    bg = bg + proj_mm_list(nb - 2)

                nsl = slice(nb * 512, nb * 512 + 512)
                osl = slice(p * N + nb * 512, p * N + nb * 512 + 512)
                ps_av_a = ps_av.tile([P, 512], F32, tag="avA")
                ps_av_b = ps_av.tile([P, 512], F32, tag="avB")
                pt_prev = None
                m_prev = None
                for m in range(NT):
                    ps_s_t = ps_s.tile([P, 1024], F32, tag="s")
                    nc.tensor.matmul(
                        ps_s_t[:, 0:512],
                        kt[0:64, m * P:(m + 1) * P],
                        qt[0:64, nsl],
                        start=True, stop=True, skip_group_check=True)
                    nc.tensor.matmul(
                        ps_s_t[:, 512:1024],
                        kt[64:128, m * P:(m + 1) * P],
                        qt[64:128, nsl],
                        start=True, stop=True, skip_group_check=True)
                    pt = pt_pool.tile([P, 1024], F16, tag="pt")
                    nc.scalar.activation(pt, ps_s_t, AF.Exp,
                                         scale=SCALE, bias=ebias)
                    # background PE work (QK of next pair / projection),
                    # front-loaded 2/iter so the last chain's eviction
                    # lands well before the pair boundary; proj waits 2
                    # iters so stage_b's muls land first
                    if p < PAIRS - 1 or m >= 2:
                        for _ in range(bg_per_iter):
                            if bgi < len(bg):
                                bg[bgi]()
                                bgi += 1
                    # AV trails scores by one m-iteration (softer PSUM reuse)
                    if pt_prev is not None:
                        self_m = m_prev
                        vbase = self_m * MBLK + 2 * p * VW
                        nc.tensor.matmul(
                            ps_av_a[0:VW, :],
                            v_sb[:, vbase: vbase + VW],
                            pt_prev[:, 0:512],
                            start=(self_m == 0), stop=False,
                            skip_group_check=True)
                        nc.tensor.matmul(
                            ps_av_b[0:VW, :],
                            v_sb[:, vbase + VW: vbase + 2 * VW],
                            pt_prev[:, 512:1024],
                            start=(self_m == 0), stop=False,
                            skip_group_check=True)
                    pt_prev, m_prev = pt, m
                vbase = m_prev * MBLK + 2 * p * VW
                nc.tensor.matmul(
                    ps_av_a[0:VW, :],
                    v_sb[:, vbase: vbase + VW],
                    pt_prev[:, 0:512],
                    start=False, stop=True, skip_group_check=True)
                nc.tensor.matmul(
                    ps_av_b[0:VW, :],
                    v_sb[:, vbase + VW: vbase + 2 * VW],
                    pt_prev[:, 512:1024],
                    start=False, stop=True, skip_group_check=True)

                # evict eagerly (2 copies free both AV banks in ~1.3us);
                # head A stays unnormalized in scra until stage_b
                scra = scr_pool.tile([VW, 512], F16, tag="sA")
                scrb = scr_pool.tile([VW, 512], F16, tag="sB")
                nc.vector.tensor_copy(scra, ps_av_a[0:VW, :])
                nc.vector.tensor_copy(scrb, ps_av_b[0:VW, :])
                # denom rows: partition 64 -> 0 shift for the Q7 broadcast
                rc0 = rc0_pool.tile([1, 1024], F16, tag="rc0")
                nc.sync.dma_start(out=rc0[0:1, 0:512], in_=scra[64:65, :])
                nc.sync.dma_start(out=rc0[0:1, 512:1024], in_=scrb[64:65, :])
                pend_a = {"rc0": rc0, "scra": scra, "scrb": scrb, "osl": osl}

            qk_cur = qk_next

        # tail: final normalizations + remaining projection work.
        # stage_a(3,3)'s reciprocal runs on DVE concurrently with chunk 2's
        # matmuls; chunk evictions ride the now-idle ScalarE.
        stage_b(pend_b)                      # (3, 2)
        stage_a(pend_a)                      # (3, 3)
        for emit in proj_mm_list(2, scalar_evict=True):
            emit()
        stage_b(pend_a)                      # (3, 3)
        for emit in proj_mm_list(3, scalar_evict=True):
            emit()


@functools.lru_cache(maxsize=1)
def build_nc():
    nc = bacc.Bacc("TRN2", target_bir_lowering=False, debug=False)
    xt_d = nc.dram_tensor("xt_local", [C, N], F16, kind="ExternalInput").ap()
    wqk_d = nc.dram_tensor("wqk", [P, 2 * PAIRS * CT * P], F16,
                           kind="ExternalInput").ap()
    wv_d = nc.dram_tensor("wv", [P, CT * DCORE], F16, kind="ExternalInput").ap()
    wp_d = nc.dram_tensor("wp", [P, PAIRS * C], F16, kind="ExternalInput").ap()
    out_d = nc.dram_tensor("out_partial", [N, C], F16, kind="ExternalOutput").ap()
    with tile.TileContext(nc) as tc:
        _kernel_body(tc, out_d, xt_d, wqk_d, wv_d, wp_d)
    nc.compile()
    return nc


def make_in_maps(x, W_qkv, W_proj):
    """Shard + repack host-side. Weights land in SBUF-partition-major
    layout so each DMA reads long contiguous DRAM runs per partition:
      wv[q, cc*512+d]           = W_v[cc*128+q, d]
      wqk[q, ((t*4+p)*8+cc)*128+f] = W_{q|k}[cc*128+q, p*128+f]
      wp[q, dc*1024+c]          = W_proj[dc*128+q, c] (per-core rows)
    """
    in_maps = []
    for core in range(NCORES):
        b, half = core // 2, core % 2
        h0 = half * HPC
        wq = W_qkv[:, 0 * C + h0 * D: 0 * C + h0 * D + DCORE].astype(np.float16)
        wk = W_qkv[:, 1 * C + h0 * D: 1 * C + h0 * D + DCORE].astype(np.float16)
        wv = W_qkv[:, 2 * C + h0 * D: 2 * C + h0 * D + DCORE].astype(np.float16)
        wp = W_proj[h0 * D: h0 * D + DCORE, :].astype(np.float16)
        # [C, DCORE] -> per-pair column blocks, partition-major
        wqk = np.stack([w.reshape(CT, P, PAIRS, P).transpose(1, 2, 0, 3)
                        for w in (wq, wk)], axis=1)      # [q, t, p, cc, f]
        in_maps.append({
            "xt_local": np.ascontiguousarray(x[b].T.astype(np.float16)),
            "wqk": np.ascontiguousarray(wqk.reshape(P, 2 * PAIRS * CT * P)),
            "wv": np.ascontiguousarray(
                wv.reshape(CT, P, DCORE).transpose(1, 0, 2).reshape(P, CT * DCORE)),
            "wp": np.ascontiguousarray(
                wp.reshape(PAIRS, P, C).transpose(1, 0, 2).reshape(P, PAIRS * C)),
        })
    return in_maps


def kernel(x, W_qkv, W_proj, b_proj, trace=False):
    x = np.asarray(x, dtype=np.float32)
    W_qkv = np.asarray(W_qkv, dtype=np.float32)
    W_proj = np.asarray(W_proj, dtype=np.float32)
    b_proj = np.asarray(b_proj, dtype=np.float32)

    nc = build_nc()
    in_maps = make_in_maps(x, W_qkv, W_proj)

    global LAST_RESULT
    res = run_bass_kernel_spmd(nc, in_maps, list(range(NCORES)), trace=trace)
    LAST_RESULT = res

    out = np.empty((B, N, C), dtype=np.float32)
    for b in range(B):
        out[b] = (res.results[2 * b]["out_partial"].astype(np.float32)
                  + res.results[2 * b + 1]["out_partial"].astype(np.float32)
                  + b_proj[None, :])
    return out


# revision 29
# speedup vs baseline: 1.2273x; 1.0406x over previous
"""Multi-head attention (B=4, N=2048, C=1024, H=16) on 8 TRN2 NeuronCores.

Sharding: core = 2*b + half handles batch b, heads half*8 .. half*8+7.
Each core computes QKV for its 8 heads, full attention for them, and a
partial projection (its 512 rows of W_proj). Host sums the two partials
per batch and adds the bias.

All matmul operands are fp16 (1 cycle/row on the PE); accumulation stays
fp32 in PSUM. The host pre-casts weights/x to fp16 and pre-transposes x.

On-chip layout is "transposed": Q^T/K^T [d, n] come straight out of the
QKV matmuls, scores are computed as S^T[m, n] so exp(S^T) = P^T is the
moving operand of the AV matmul. Row sums of P ride along as a 65th
stationary column of ones.

v2 pipeline (vs the v1 baseline at 577us):
- Softmax normalization is DEFERRED by two nb-iterations and runs entirely
  off the tensor queue: DVE reciprocal of the raw row sums, a GPSIMD
  partition_broadcast replacing the K=1 broadcast matmuls, two DVE
  multiplies, and the head-B partition-shift DMA. The PE never waits on
  the (slow, 8 cyc/elem) DVE reciprocal.
- All weights are prefetched up front; the exp activation table is
  preloaded during the input DMA.
- Q^T/K^T for pair p+1 are computed one matmul per m-iteration inside
  pair p's ACT-paced m-loop (PE slack), as is the projection during the
  last pair's m-loops.
- Output is fp16 (halves the tail DMA); host accumulates in fp32.
"""

import functools
from contextlib import ExitStack

import numpy as np

import concourse.bass as bass
import concourse.tile as tile
from concourse import bacc, mybir
from concourse.bass_utils import run_bass_kernel_spmd

F32 = mybir.dt.float32
F16 = mybir.dt.float16
AF = mybir.ActivationFunctionType

B, N, C = 4, 2048, 1024
H, D = 16, 64
P = 128
NCORES = 8
HPC = 8            # heads per core
PAIRS = HPC // 2   # 4
DCORE = HPC * D    # 512 attention columns per core
SCALE = float(H) ** -0.5  # 0.25 (faithful to reference: num_heads**-0.5)
EXP_BIAS = -5.0    # exp(scale*s + bias): cancels in softmax, keeps fp16 range
NB = N // 512      # 4 n blocks
NT = N // P        # 16 tiles of 128
CT = C // P        # 8 contraction chunks
VW = D + 1         # V columns per head incl. the ones column (row sums)
MBLK = HPC * VW    # 520 v_sb columns per m-tile

LAST_RESULT = None  # BassKernelResults of the most recent run (for test.py)


def _kernel_body(tc, out_d, xt_d, wqk_d, wv_d, wp_d):
    nc = tc.nc
    # DRAM scratch for the softmax row sums / reciprocals (see stage_a)
    ds_sums = nc.dram_tensor("ds_sums", [NB * PAIRS, 1024], F16,
                             kind="Internal").ap()
    ds_recip = nc.dram_tensor("ds_recip", [NB * PAIRS, 1024], F16,
                              kind="Internal").ap()
    with ExitStack() as ctx:
        const = ctx.enter_context(tc.tile_pool(name="const", bufs=1))
        ones_f = const.tile([P, P], F32)
        nc.vector.memset(ones_f, 1.0)
        ebias = const.tile([P, 1], F32)
        nc.vector.memset(ebias, EXP_BIAS)
        warm = const.tile([P, 1], F16)

        # attT: pair p occupies cols [p*N, (p+1)*N); partitions = 2 heads x 64
        attT_pool = ctx.enter_context(tc.tile_pool(name="attT", bufs=1))
        attT = attT_pool.tile([P, PAIRS * N], F16)

        # PSUM: ps_mm 2 (V/QK/proj chains) + ps_s 2x2 (scores) + av 2 = 8 banks
        ps_mm = ctx.enter_context(tc.tile_pool(name="ps_mm", bufs=2, space="PSUM"))
        ps_s = ctx.enter_context(tc.tile_pool(name="ps_s", bufs=2, space="PSUM"))
        ps_av = ctx.enter_context(tc.tile_pool(name="ps_av", bufs=1, space="PSUM"))

        # ---- Phase 0: prefetch everything; preload the exp table ----
        w_pool = ctx.enter_context(tc.tile_pool(name="weights", bufs=1))
        wv_sb = w_pool.tile([P, CT * DCORE], F16)
        xt_pool = ctx.enter_context(tc.tile_pool(name="xt", bufs=1))
        xt = xt_pool.tile([P, CT * N], F16)
        # wqk layout: tensor t (0=q, 1=k), pair p at [(t*PAIRS+p) * CT*P, +CT*P)
        wqk_sb = w_pool.tile([P, 2 * PAIRS * CT * P], F16)
        wp_sb = w_pool.tile([P, PAIRS * C], F16)

        # All weights are host-repacked so every partition's data is one
        # contiguous DRAM run (8-32KB lines). Inputs split across BOTH
        # HWDGE queues (SP + Activation) for ~2x landing bandwidth; wv and
        # xt chunks first (they gate the V phase), wqk/wp trail.
        half = CT * DCORE // 2
        nc.sync.dma_start(out=wv_sb[:, 0:half], in_=wv_d[:, 0:half])
        nc.scalar.dma_start(out=wv_sb[:, half:], in_=wv_d[:, half:])
        for j in range(CT):
            eng = nc.sync if j % 2 == 0 else nc.scalar
            eng.dma_start(out=xt[:, j * N:(j + 1) * N],
                          in_=xt_d[j * P:(j + 1) * P, :])
        nc.sync.dma_start(out=wqk_sb, in_=wqk_d)
        nc.scalar.dma_start(out=wp_sb, in_=wp_d)
        # exp table preload (~2.7us) overlaps the input DMA
        nc.scalar.activation(warm, ebias, AF.Exp)

        # V storage: m-tile m at cols [m*MBLK, ...); head hl at
        # [m*MBLK + hl*VW, +D], then a ones column (for row sums)
        v_pool = ctx.enter_context(tc.tile_pool(name="v", bufs=1))
        v_sb = v_pool.tile([P, NT * MBLK], F16)
        ones_cols = v_sb.rearrange("q (g k) -> q g k", k=VW)[:, :, D:VW]
        nc.vector.tensor_copy(
            ones_cols, ones_f.rearrange("q (g k) -> q g k", k=1))

        # ---- Phase B1: V for all 8 heads ----
        # 8 chains open at once (borrowing the idle ps_s/ps_av banks) so
        # 7/8 of the matmuls pre-run while the xt DMA is still landing;
        # only each chain's last chunk gates on the final xt arrival.
        def v_evict(psv_half, m):
            nc.vector.tensor_copy(
                v_sb[:, m * MBLK:(m + 1) * MBLK].rearrange(
                    "q (h k) -> q h k", k=VW)[:, :, 0:D],
                psv_half.rearrange("q (h k) -> q h k", k=D))

        for g in range(2):
            slots = [
                ps_mm.tile([P, DCORE], F32, tag="mm", name=f"vmm0_{g}"),
                ps_mm.tile([P, DCORE], F32, tag="mm", name=f"vmm1_{g}"),
                ps_av.tile([P, DCORE], F32, tag="avA", name=f"vavA_{g}"),
                ps_av.tile([P, DCORE], F32, tag="avB", name=f"vavB_{g}"),
                ps_s.tile([P, 2 * DCORE], F32, tag="s", name=f"vs0_{g}"),
                ps_s.tile([P, 2 * DCORE], F32, tag="s", name=f"vs1_{g}"),
            ]

            def v_dst(j):
                if j < 4:
                    return slots[j]
                big = slots[4 + (j - 4) // 2]
                h = (j - 4) % 2
                return big[:, h * DCORE:(h + 1) * DCORE]

            for cc in range(CT):
                for j in range(8):
                    m = 8 * g + j
                    nc.tensor.matmul(
                        v_dst(j),
                        xt[:, cc * N + m * P: cc * N + (m + 1) * P],
                        wv_sb[:, cc * DCORE:(cc + 1) * DCORE],
                        start=(cc == 0), stop=(cc == CT - 1),
                        skip_group_check=True)
            for j in range(8):
                v_evict(v_dst(j), 8 * g + j)

        # ---- QK + attention, software-pipelined across pairs ----
        qt_pool = ctx.enter_context(tc.tile_pool(name="qt", bufs=2))
        kt_pool = ctx.enter_context(tc.tile_pool(name="kt", bufs=2))
        pt_pool = ctx.enter_context(tc.tile_pool(name="pt", bufs=4))
        scr_pool = ctx.enter_context(tc.tile_pool(name="scr", bufs=3))
        rsp_pool = ctx.enter_context(tc.tile_pool(name="rsp", bufs=3))
        rbt_pool = ctx.enter_context(tc.tile_pool(name="rbt", bufs=3))
        stage_pool = ctx.enter_context(tc.tile_pool(name="stage", bufs=3))

        def new_qk_tiles():
            qt = qt_pool.tile([P, N], F16, tag="qt")
            kt = kt_pool.tile([P, N], F16, tag="kt")
            return qt, kt

        def qk_mm_list(p, qt, kt):
            """Flat (chain-structured) QK matmul emitters for pair p.

            8 chains of 8: chain j = (tensor t=j//4, nb j%4); each chain
            accumulates CT chunks into a ps_mm tile then evicts to qt/kt.
            Returns a list of 64 closures, one matmul (+ trailing copy) each.
            """
            state = {}

            def make(j, cc):
                def emit():
                    t, nbq = j // 4, j % 4
                    if cc == 0:
                        state[j] = ps_mm.tile([P, 512], F32, tag="mm", name=f"psq{j}")
                    psq = state[j]
                    wbase = (t * PAIRS + p) * CT * P
                    nc.tensor.matmul(
                        psq,
                        wqk_sb[:, wbase + cc * P: wbase + (cc + 1) * P],
                        xt[:, cc * N + nbq * 512: cc * N + nbq * 512 + 512],
                        start=(cc == 0), stop=(cc == CT - 1),
                        skip_group_check=True)
                    if cc == CT - 1:
                        dst = qt if t == 0 else kt
                        nc.vector.tensor_copy(
                            dst[:, nbq * 512:(nbq + 1) * 512], psq)
                        del state[j]
                return emit
            return [make(j, cc) for j in range(8) for cc in range(CT)]

        def proj_mm_list(k, scalar_evict=False):
            """Projection chunk k: i-tiles 4k..4k+3, both co halves.

            Chains of 4 dc-matmuls into ps_mm, then fp16 stage + DMA out.
            32 closures, one matmul (+ trailing evict) each. At the tail
            the exp stream is over, so the stage copies route to the idle
            ScalarE instead of the busy DVE (scalar_evict=True).
            """
            state = {}

            def make(i, co, dc):
                def emit():
                    key = (i, co)
                    if dc == 0:
                        state[key] = ps_mm.tile([P, 512], F32, tag="mm", name=f"psp{i}_{co}")
                    psp = state[key]
                    nc.tensor.matmul(
                        psp,
                        attT[:, dc * N + i * P: dc * N + (i + 1) * P],
                        wp_sb[:, dc * C + co * 512: dc * C + co * 512 + 512],
                        start=(dc == 0), stop=(dc == PAIRS - 1),
                        skip_group_check=True)
                    if dc == PAIRS - 1:
                        st = stage_pool.tile([P, 512], F16, tag="st")
                        if scalar_evict:
                            nc.scalar.copy(st, psp)
                        else:
                            nc.vector.tensor_copy(st, psp)
                        nc.sync.dma_start(
                            out=out_d[i * P:(i + 1) * P,
                                      co * 512: co * 512 + 512],
                            in_=st)
                        del state[key]
                return emit
            return [make(i, co, dc)
                    for i in range(4 * k, 4 * k + 4)
                    for co in range(2)
                    for dc in range(PAIRS)]

        def stage_a(pend):
            """Deferred norm 1/2: reciprocal + broadcast of the row sums.

            The sums bounce through DRAM to respread [1,1024] -> [128,8]
            (8 elems/lane makes the 8-cyc/elem DVE reciprocal ~0.2us, not
            8.5us), then back, then a DRAM broadcast-read fans the
            reciprocals to 64 partitions. All five DMAs share the SP
            queue, so they execute in order without extra semaphores."""
            w = pend["w"]
            rsp = rsp_pool.tile([P, CT], F16, tag="rsp")
            nc.sync.dma_start(
                out=rsp,
                in_=ds_sums[w:w + 1, :].rearrange("a (q b) -> (a q) b", q=P))
            with nc.allow_low_precision(
                    reason="softmax recip rounding is benign"):
                nc.vector.reciprocal(rsp, rsp)
            nc.sync.dma_start(
                out=ds_recip[w:w + 1, :].rearrange("a (q b) -> (a q) b", q=P),
                in_=rsp)
            rbt = rbt_pool.tile([64, 1024], F16, tag="rbt")
            nc.sync.dma_start(
                out=rbt, in_=ds_recip[w:w + 1, :].to_broadcast((64, 1024)))
            pend["rbt"] = rbt

        def stage_b(pend):
            """Deferred norm 2/2: scale by the broadcast reciprocals
            (head A lands in attT directly), then shift head B's rows
            (partitions 0-63) up to partitions 64-127."""
            rbt, scra, scrb, osl = (pend["rbt"], pend["scra"],
                                    pend["scrb"], pend["osl"])
            nc.vector.tensor_mul(attT[0:64, osl], scra[0:64, :],
                                 rbt[:, 0:512])
            nc.vector.tensor_mul(scrb[0:64, :], scrb[0:64, :],
                                 rbt[:, 512:1024])
            nc.sync.dma_start(out=attT[64:128, osl], in_=scrb[0:64, :])

        # QK for pair 0 runs serially (nothing to hide it under)
        qk_cur = new_qk_tiles()
        for emit in qk_mm_list(0, *qk_cur):
            emit()

        pend_a = None   # evicted, awaiting stage_a
        pend_b = None   # awaiting stage_b
        qk_next = None
        for p in range(PAIRS):
            qt, kt = qk_cur
            if p < PAIRS - 1:
                qk_next = new_qk_tiles()
                bg = qk_mm_list(p + 1, *qk_next)   # 64 mms: 2/iter, front-half
                bg_per_iter = 2
            else:
                bg = []                            # filled per-nb with proj
                bg_per_iter = 3
            bgi = 0

            for nb in range(NB):
                if pend_b is not None:
                    stage_b(pend_b)
                    pend_b = None
                if pend_a is not None:
                    stage_a(pend_a)
                    pend_b = pend_a
                    pend_a = None
                if p == PAIRS - 1 and nb >= 2:
                    # proj chunk nb-2 became ready with stage_b(p, nb-2)
                    bg = bg + proj_mm_list(nb - 2)

                nsl = slice(nb * 512, nb * 512 + 512)
                osl = slice(p * N + nb * 512, p * N + nb * 512 + 512)
                ps_av_a = ps_av.tile([P, 512], F32, tag="avA")
                ps_av_b = ps_av.tile([P, 512], F32, tag="avB")
                pt_prev = None
                m_prev = None
                for m in range(NT):
                    ps_s_t = ps_s.tile([P, 1024], F32, tag="s")
                    nc.tensor.matmul(
                        ps_s_t[:, 0:512],
                        kt[0:64, m * P:(m + 1) * P],
                        qt[0:64, nsl],
                        start=True, stop=True, skip_group_check=True)
                    nc.tensor.matmul(
                        ps_s_t[:, 512:1024],
                        kt[64:128, m * P:(m + 1) * P],
                        qt[64:128, nsl],
                        start=True, stop=True, skip_group_check=True)
                    pt = pt_pool.tile([P, 1024], F16, tag="pt")
                    nc.scalar.activation(pt, ps_s_t, AF.Exp,
                                         scale=SCALE, bias=ebias)
                    # background PE work (QK of next pair / projection),
                    # front-loaded 2/iter so the last chain's eviction
                    # lands well before the pair boundary; proj waits 2
                    # iters so stage_b's muls land first
                    if p < PAIRS - 1 or m >= 2:
                        for _ in range(bg_per_iter):
                            if bgi < len(bg):
                                bg[bgi]()
                                bgi += 1
                    # AV trails scores by one m-iteration (softer PSUM reuse)
                    if pt_prev is not None:
                        self_m = m_prev
                        vbase = self_m * MBLK + 2 * p * VW
                        nc.tensor.matmul(
                            ps_av_a[0:VW, :],
                            v_sb[:, vbase: vbase + VW],
                            pt_prev[:, 0:512],
                            start=(self_m == 0), stop=False,
                            skip_group_check=True)
                        nc.tensor.matmul(
                            ps_av_b[0:VW, :],
                            v_sb[:, vbase + VW: vbase + 2 * VW],
                            pt_prev[:, 512:1024],
                            start=(self_m == 0), stop=False,
                            skip_group_check=True)
                    pt_prev, m_prev = pt, m
                vbase = m_prev * MBLK + 2 * p * VW
                nc.tensor.matmul(
                    ps_av_a[0:VW, :],
                    v_sb[:, vbase: vbase + VW],
                    pt_prev[:, 0:512],
                    start=False, stop=True, skip_group_check=True)
                nc.tensor.matmul(
                    ps_av_b[0:VW, :],
                    v_sb[:, vbase + VW: vbase + 2 * VW],
                    pt_prev[:, 512:1024],
                    start=False, stop=True, skip_group_check=True)

                # evict eagerly (2 copies free both AV banks in ~1.3us);
                # head A stays unnormalized in scra until stage_b
                w = p * NB + nb
                scra = scr_pool.tile([VW, 512], F16, tag="sA")
                scrb = scr_pool.tile([VW, 512], F16, tag="sB")
                nc.vector.tensor_copy(scra, ps_av_a[0:VW, :])
                nc.vector.tensor_copy(scrb, ps_av_b[0:VW, :])
                # denom rows out to the DRAM scratch (respread in stage_a)
                nc.sync.dma_start(out=ds_sums[w:w + 1, 0:512],
                                  in_=scra[64:65, :])
                nc.sync.dma_start(out=ds_sums[w:w + 1, 512:1024],
                                  in_=scrb[64:65, :])
                pend_a = {"w": w, "scra": scra, "scrb": scrb, "osl": osl}

            qk_cur = qk_next

        # tail: final normalizations + remaining projection work.
        # stage_a(3,3)'s reciprocal runs on DVE concurrently with chunk 2's
        # matmuls; chunk evictions ride the now-idle ScalarE.
        stage_b(pend_b)                      # (3, 2)
        stage_a(pend_a)                      # (3, 3)
        for emit in proj_mm_list(2, scalar_evict=True):
            emit()
        stage_b(pend_a)                      # (3, 3)
        for emit in proj_mm_list(3, scalar_evict=True):
            emit()


@functools.lru_cache(maxsize=1)
def build_nc():
    nc = bacc.Bacc("TRN2", target_bir_lowering=False, debug=False)
    xt_d = nc.dram_tensor("xt_local", [C, N], F16, kind="ExternalInput").ap()
    wqk_d = nc.dram_tensor("wqk", [P, 2 * PAIRS * CT * P], F16,
                           kind="ExternalInput").ap()
    wv_d = nc.dram_tensor("wv", [P, CT * DCORE], F16, kind="ExternalInput").ap()
    wp_d = nc.dram_tensor("wp", [P, PAIRS * C], F16, kind="ExternalInput").ap()
    out_d = nc.dram_tensor("out_partial", [N, C], F16, kind="ExternalOutput").ap()
    with tile.TileContext(nc) as tc:
        _kernel_body(tc, out_d, xt_d, wqk_d, wv_d, wp_d)
    nc.compile()
    return nc


def make_in_maps(x, W_qkv, W_proj):
    """Shard + repack host-side. Weights land in SBUF-partition-major
    layout so each DMA reads long contiguous DRAM runs per partition:
      wv[q, cc*512+d]           = W_v[cc*128+q, d]
      wqk[q, ((t*4+p)*8+cc)*128+f] = W_{q|k}[cc*128+q, p*128+f]
      wp[q, dc*1024+c]          = W_proj[dc*128+q, c] (per-core rows)
    """
    in_maps = []
    for core in range(NCORES):
        b, half = core // 2, core % 2
        h0 = half * HPC
        wq = W_qkv[:, 0 * C + h0 * D: 0 * C + h0 * D + DCORE].astype(np.float16)
        wk = W_qkv[:, 1 * C + h0 * D: 1 * C + h0 * D + DCORE].astype(np.float16)
        wv = W_qkv[:, 2 * C + h0 * D: 2 * C + h0 * D + DCORE].astype(np.float16)
        wp = W_proj[h0 * D: h0 * D + DCORE, :].astype(np.float16)
        # [C, DCORE] -> per-pair column blocks, partition-major
        wqk = np.stack([w.reshape(CT, P, PAIRS, P).transpose(1, 2, 0, 3)
                        for w in (wq, wk)], axis=1)      # [q, t, p, cc, f]
        in_maps.append({
            "xt_local": np.ascontiguousarray(x[b].T.astype(np.float16)),
            "wqk": np.ascontiguousarray(wqk.reshape(P, 2 * PAIRS * CT * P)),
            "wv": np.ascontiguousarray(
                wv.reshape(CT, P, DCORE).transpose(1, 0, 2).reshape(P, CT * DCORE)),
            "wp": np.ascontiguousarray(
                wp.reshape(PAIRS, P, C).transpose(1, 0, 2).reshape(P, PAIRS * C)),
        })
    return in_maps


def kernel(x, W_qkv, W_proj, b_proj, trace=False):
    x = np.asarray(x, dtype=np.float32)
    W_qkv = np.asarray(W_qkv, dtype=np.float32)
    W_proj = np.asarray(W_proj, dtype=np.float32)
    b_proj = np.asarray(b_proj, dtype=np.float32)

    nc = build_nc()
    in_maps = make_in_maps(x, W_qkv, W_proj)

    global LAST_RESULT
    res = run_bass_kernel_spmd(nc, in_maps, list(range(NCORES)), trace=trace)
    LAST_RESULT = res

    out = np.empty((B, N, C), dtype=np.float32)
    for b in range(B):
        out[b] = (res.results[2 * b]["out_partial"].astype(np.float32)
                  + res.results[2 * b + 1]["out_partial"].astype(np.float32)
                  + b_proj[None, :])
    return out
